# revision 31
# baseline (speedup 1.0000x reference)
"""Trainium2 Bass kernel for single-head causal attention.

Problem: x [4, 4096, 1024], Wk/Wq/Wv [64, 1024] -> out [4, 4096, 64]
  k/q/v = x @ W^T;  out = softmax(causal(q k^T / 8)) @ v

Sharding (8 cores): 2 cores per batch, striped (parity) sequence-parallel
over T. Core c handles batch c//2, query rows of parity c%2. Each core
computes K/V for both parities and full attention for its 2048 query rows.

Structure (v3):
- Row-tiled scores: K for parity0 lives in SBUF partitions 0:64, parity1
  in 64:128; Q is duplicated into both partition halves. The two scores
  matmuls of a (kb, parity-pair) then occupy disjoint PE row-groups
  (tile_position (0,0)/(64,0)) and stream concurrently -- one column
  read of qTd feeds both -- halving scores PE time vs zero-padding.
- exp chunks are [128, 1024] PSUM tiles (2 banks): per-parity blocks are
  first-fit packed into 512-col banks (a matmul output may not cross a
  bank) and mirrored across parities, so each chunk holds one bank-pair;
  chunks alternate two pool tags for double buffering.
- Causal masking of diagonal blocks is fused into scores as an additive
  (-30000) matrix applied via a PE matmul with identity moving operand.
- Startup: the K|Q weights ship first (tiny DMA), the first x tiles are
  split into half-tile DMAs so the first projection starts ~4us earlier;
  remaining consts ship as one contiguous blob; x is pre-swizzled on
  host so every x DMA is a contiguous 8KB-per-partition transfer.
- V gets a ones column so the softmax denominator accumulates in the
  same PSUM tile as the output; host does the final divide.
- Projection work for tile t+1 is interleaved between attention chunks
  of q-tile t so the PE has work while ACT runs exp.
- PSUM: psP 2x1 bank (proj accum + V transposes), psS 2+2 banks
  (score chunks), psO 2 banks (split output accumulators) = 8 banks.

v4 changes:
- AV matmuls are row-tiled: keys 0:64 accumulate into o_ps[:, 0:512]
  (PE rows 0:64), keys 64:128 into o_ps[:, 512:1024] (rows 64:128).
  The two streams share each p_sb column read and run concurrently,
  halving AV PE time; a single DVE add fuses the two accumulators into
  o_sb (replacing the old copy).
- Mask matmuls are row-tiled via identity blocks: mask cols 0:64 from
  contraction rows 0:64 (lhsT=msk[0:64,:], rhs=I64 at parts 0:64) and
  cols 64:128 from rows 64:128, concurrent, halving mask PE time.
- ~8 warm-up matmuls on a memset SBUF tile run during the initial DMA
  latency window so the PE HAM clock gate reaches 2.4 GHz before the
  first projection (was cold 1.2 GHz until ~20us).
"""

import numpy as np

B, T, C, H = 4, 4096, 1024, 64
NCORES = 8
TL = T // 2          # local query rows per core
NB = TL // 128       # 16 local 128-row blocks
NQT = TL // 512      # 4 q-tiles of 512
NE = C // 128        # 8 contraction chunks
SCALE = 1.0 / np.sqrt(H)
NEG = -30000.0

# consts: cmask carries masks+identity (tiny, shipped first on the scalar
# HWDGE queue); cw carries the KV/V projection weights.
OFF_M0, OFF_M1, OFF_ID = 0, 128, 256
CM_COLS = 384

_CACHE = {}


def _unit_plan(qt):
    """FFD-pack one parity's (kb) blocks into 512-col banks; mirror across
    parities. Returns a list of units; each unit is a list of
    (s, kb, qs, w, off) with s=0 blocks at off and s=1 mirrored at 512+off,
    pairs (s0, s1) of the same kb adjacent.
    """
    blocks = []
    nkb = 4 * qt + 4
    for kb in range(nkb):
        qs = max(0, (kb - 4 * qt) * 128)
        blocks.append((kb, qs, 512 - qs))
    blocks.sort(key=lambda b: -b[2])
    banks = []
    for (kb, qs, w) in blocks:
        for bnk in banks:
            if bnk[0] + w <= 512:
                bnk[1].append((kb, qs, w, bnk[0]))
                bnk[0] += w
                break
        else:
            banks.append([w, [(kb, qs, w, 0)]])
    units = []
    for fill, entries in banks:
        unit = []
        for (kb, qs, w, off) in entries:
            unit.append((0, kb, qs, w, off))
            unit.append((1, kb, qs, w, 512 + off))
        units.append((unit, 512 + fill))
    return units


def _build_program():
    import concourse.bacc as bacc
    import concourse.tile as tile
    import concourse.mybir as mybir

    F32 = mybir.dt.float32
    F16 = mybir.dt.float16
    EXP = mybir.ActivationFunctionType.Exp

    nc = bacc.Bacc("TRN2", target_bir_lowering=False, debug=False,
                   num_devices=NCORES)

    xt_ap = nc.dram_tensor("xt0", [128, NQT, NE, 512], F16,
                           kind="ExternalInput").ap()
    wkq_ap = nc.dram_tensor("wkq", [128, NE * 128], F16,
                            kind="ExternalInput").ap()
    cm_ap = nc.dram_tensor("cmask", [128, CM_COLS], F16,
                           kind="ExternalInput").ap()
    wv_ap = nc.dram_tensor("wv", [128, 512], F16,
                           kind="ExternalInput").ap()
    outT_ap = nc.dram_tensor("outT", [H + 1, TL], F32, kind="ExternalOutput").ap()
    # pair AllGather groups: cores (2b, 2b+1) share batch b; within a
    # group the rank order equals the key-parity order, so gathered slot
    # p is parity-p data on BOTH cores (rank-independent addressing).
    PAIRS = [[2 * b, 2 * b + 1] for b in range(NCORES // 2)]

    with tile.TileContext(nc) as tc:
        with (
            tc.tile_pool(name="consts", bufs=1) as consts,
            tc.tile_pool(name="persist", bufs=1) as persist,
            tc.tile_pool(name="xin", bufs=8) as xin,
            tc.tile_pool(name="vt", bufs=4) as vtp,
            tc.tile_pool(name="pb", bufs=4) as pbp,
            tc.tile_pool(name="ob", bufs=2) as obp,
            tc.tile_pool(name="psP", bufs=3, space="PSUM") as psP,
            tc.tile_pool(name="psS", bufs=1, space="PSUM") as psS,
            tc.tile_pool(name="psO", bufs=1, space="PSUM") as psO,
            tc.tile_pool(name="exd", bufs=1, space="DRAM") as exd,
        ):
            # ---- weight DMAs on the scalar HWDGE queue (runs concurrently
            # with the x queue on sync): masks first (tiny, unblocks the
            # qt0 diag chain), then K|Q weights, then K|V / V weights.
            cmsk = consts.tile([128, CM_COLS], F16)
            nc.scalar.dma_start(cmsk[:], cm_ap[:])
            wkqf = consts.tile([128, NE * 128], F16)
            nc.scalar.dma_start(wkqf[:], wkq_ap[:])
            wvf = consts.tile([128, 512], F16)
            nc.scalar.dma_start(wvf[:], wv_ap[:])

            # warm tile for ACT table preload (no DMA dependency)
            warm_sb = consts.tile([128, 8], F16)
            escr = consts.tile([128, 1], F16)
            nc.vector.memset(warm_sb[:], 1.0)
            nc.scalar.activation(escr[:], warm_sb[:, 0:1], EXP, scale=1.0)

            # PE HAM warm-up: dummy matmuls on a memset tile fill the
            # initial DMA-latency window so the clock gate opens (2.4GHz)
            # before the first real projection.
            NDUM = 10
            warm_mm = consts.tile([128, 512], F16)
            nc.vector.memset(warm_mm[:], 0.001)
            dum_ps = psP.tile([128, 512], F32, tag="p", name="dummy")
            for _ in range(NDUM):
                nc.tensor.matmul(dum_ps[:], warm_mm[:, 0:128], warm_mm[:],
                                 start=True, stop=True)

            # ---- x DMAs on the sync HWDGE queue (own parity only) ----
            x_sb = {}

            def emit_xdma(t, split):
                xs = xin.tile([128, NE, 512], F16, tag="x", name=f"x{t}")
                if split:
                    nc.sync.dma_start(out=xs[:, 0:2, :], in_=xt_ap[:, t, 0:2])
                    nc.sync.dma_start(out=xs[:, 2:4, :], in_=xt_ap[:, t, 2:4])
                    nc.sync.dma_start(out=xs[:, 4:8, :], in_=xt_ap[:, t, 4:8])
                else:
                    nc.sync.dma_start(out=xs[:], in_=xt_ap[:, t])
                x_sb[t] = xs

            emit_xdma(0, True)
            for t in range(1, NQT):
                emit_xdma(t, False)

            wkq_sb = wkqf.rearrange("p (e j) -> p e j", e=NE)
            wv_sb = wvf.rearrange("p (e j) -> p e j", e=NE)
            m0_sb = cmsk[:, OFF_M0:OFF_M0 + 128]
            m1_sb = cmsk[:, OFF_M1:OFF_M1 + 128]
            ident_sb = cmsk[:, OFF_ID:OFF_ID + 128]

            # ---- persistent K/Q/V ----
            # kTrt rows 0:64 = K parity0, rows 64:128 = K parity1 (BOTH
            # arrive via the pair AllGather readback -- own K round-trips
            # so addressing is parity-indexed and rank-independent).
            # qTd: Q duplicated in both partition halves (local).
            kTrt = persist.tile([128, TL], F16)
            qTd = persist.tile([128, TL], F16)
            vcomb = persist.tile([128, NB, 2, H + 1], F16)
            nc.vector.memset(vcomb[:, :, :, H:H + 1], 1.0)

            # ---- projection groups (own parity only) + exchange ----
            # stage layout [128, 512] f16: cols 0:256 = K packed (cols
            # 0:256 of the 512-wide K in rows 0:64, cols 256:512 in rows
            # 64:128); cols 256:512 = V blocks 4t..4t+3 at 64-col slots.
            def g_kq(t):
                cols = slice(t * 512, (t + 1) * 512)
                x0 = x_sb[t]
                kq_ps = psP.tile([128, 512], F32, tag="p", name=f"kq{t}")
                for e in range(NE):
                    nc.tensor.matmul(kq_ps[:], wkq_sb[:, e, :], x0[:, e, :],
                                     start=(e == 0), stop=(e == NE - 1))
                stg = vtp.tile([128, 512], F16, tag="stg", name=f"stg{t}")
                nc.vector.tensor_copy(stg[0:64, 0:256], kq_ps[0:64, 0:256])
                nc.vector.tensor_copy(stg[64:128, 0:256], kq_ps[0:64, 256:512])
                nc.vector.tensor_copy(qTd[0:64, cols], kq_ps[64:128, :])
                nc.vector.tensor_copy(qTd[64:128, cols], kq_ps[64:128, :])
                return stg

            def g_v0(t):
                # The two 64-channel halves go to disjoint PSUM partition
                # ranges (col groups h0/h64); interleaving their emission
                # lets each e-pair run concurrently on the PE (full
                # contraction, disjoint col groups).
                x0 = x_sb[t]
                v0_ps = psP.tile([128, 256], F32, tag="p", name=f"v0{t}")
                for e in range(NE):
                    nc.tensor.matmul(v0_ps[0:64, :], wv_sb[:, e, :],
                                     x0[:, e, 0:256],
                                     start=(e == 0), stop=(e == NE - 1))
                    nc.tensor.matmul(v0_ps[64:128, :], wv_sb[:, e, :],
                                     x0[:, e, 256:512],
                                     start=(e == 0), stop=(e == NE - 1))
                vst0 = vtp.tile([128, 256], F16, tag="v", name=f"vs0_{t}")
                nc.vector.tensor_copy(vst0[:], v0_ps[:])
                return vst0

            def g_tp(t, vst0, stg):
                # transpose own V into the stage: block 4t+u from tp cols
                # 0:64, block 4t+2+u from cols 64:128
                for u in range(2):
                    tp = psP.tile([128, 128], F16, tag="p",
                                  name=f"tp{t}_{u}")
                    nc.tensor.transpose(tp[:], vst0[:, u * 128:(u + 1) * 128],
                                        ident_sb[:])
                    nc.vector.tensor_copy(
                        stg[:, 256 + u * 64:256 + (u + 1) * 64], tp[:, 0:64])
                    nc.vector.tensor_copy(
                        stg[:, 256 + (2 + u) * 64:256 + (3 + u) * 64],
                        tp[:, 64:128])

            def g_exch(t, stg):
                # SBUF -> DRAM -> pair AllGather -> unpack both parity
                # slots into kTrt / vcomb (static addresses on all cores).
                exo = exd.tile([128, 512], F16, tag=f"exo{t}",
                               name=f"exo{t}")
                exg = exd.tile([256, 512], F16, tag=f"exg{t}",
                               name=f"exg{t}")
                nc.sync.dma_start(out=exo[:], in_=stg[:])
                nc.gpsimd.collective_compute(
                    "AllGather", mybir.AluOpType.bypass,
                    replica_groups=PAIRS,
                    ins=[exo[:].opt()], outs=[exg[:].opt()])
                for s in (0, 1):
                    base = 128 * s
                    nc.sync.dma_start(
                        out=kTrt[64 * s:64 * s + 64, t * 512:t * 512 + 256],
                        in_=exg[base:base + 64, 0:256])
                    nc.sync.dma_start(
                        out=kTrt[64 * s:64 * s + 64,
                                 t * 512 + 256:(t + 1) * 512],
                        in_=exg[base + 64:base + 128, 0:256])
                    nc.sync.dma_start(
                        out=vcomb[:, 4 * t:4 * (t + 1), s, 0:H],
                        in_=exg[base:base + 128, 256:512].rearrange(
                            "p (u h) -> p u h", u=4))

            def proj_groups(t):
                holder = {}

                def a():
                    holder["stg"] = g_kq(t)

                def c():
                    holder["v0"] = g_v0(t)

                def d():
                    g_tp(t, holder["v0"], holder["stg"])
                    g_exch(t, holder["stg"])

                return [a, c, d]

            # ---- attention: global exp-ahead pipeline ----
            # scores+exp stream runs ahead; AV matmuls trail by LAG chunks
            # so the ACT engine is never starved waiting for a q-tile's AVs.
            LAG = 2
            tag_state = {"flip": False}
            qchunks = {qt: _unit_plan(qt) for qt in range(NQT)}
            glob = [(qt, ci) for qt in range(NQT)
                    for ci in range(len(qchunks[qt]))]
            # slot tile t's projections+exchange early inside q-tile t-1 so
            # the AllGather round-trip completes before q-tile t's scores
            # need the gathered K/V.
            slot_groups = {}
            for t in range(1, NQT):
                a, c, d_ = proj_groups(t)
                prev_n = len(qchunks[t - 1])
                slot_groups.setdefault(
                    glob.index((t - 1, min(0, prev_n - 1))), []).append(a)
                slot_groups.setdefault(
                    glob.index((t - 1, min(1, prev_n - 1))), []).append(c)
                slot_groups.setdefault(
                    glob.index((t - 1, min(2, prev_n - 1))), []).append(d_)

            o_state = {}

            def emit_scores(qt, ci):
                ch, width = qchunks[qt][ci]
                tag = "sA" if tag_state["flip"] else "sB"
                tag_state["flip"] = not tag_state["flip"]
                sg = psS.tile([128, width], F32, tag=tag,
                              name=f"sg{qt}_{ci}", padded_shape=[128, 1024])
                for j in range(0, len(ch), 2):
                    for (s, kb, qs, w, pos) in ch[j:j + 2]:
                        rows = slice(64 * s, 64 * s + 64)
                        diag = kb >= 4 * qt
                        nc.tensor.matmul(
                            sg[:, pos:pos + w],
                            kTrt[rows, kb * 128:(kb + 1) * 128],
                            qTd[rows, qt * 512 + qs:(qt + 1) * 512],
                            start=True, stop=not diag)
                    pair = [b for b in ch[j:j + 2] if b[1] >= 4 * qt]
                    if pair and False:  # bisect: mask pairing disabled
                        # Row-tiled mask adds. Each mask splits into a
                        # rows-0:64 half (cols 0:64 via I64 at parts 0:64)
                        # and a rows-64:128 half (cols 64:128). Pair the
                        # s0 A-half with the s1 B-half (and vice versa) so
                        # each concurrent pair hits disjoint PE row groups
                        # AND disjoint PSUM banks (s0 is in bank 0, s1 in
                        # bank 1 of the chunk).
                        def mask_mm(s, pos, half):
                            msk = m0_sb if s == 0 else m1_sb
                            if half == 0:
                                nc.tensor.matmul(sg[:, pos:pos + 64],
                                                 msk[0:64, :],
                                                 ident_sb[0:64, 0:64],
                                                 start=False, stop=True)
                            else:
                                nc.tensor.matmul(sg[:, pos + 64:pos + 128],
                                                 msk[64:128, :],
                                                 ident_sb[64:128, 64:128],
                                                 start=False, stop=True)

                        (s0, _, _, _, p0) = pair[0]
                        if len(pair) == 2:
                            (s1, _, _, _, p1) = pair[1]
                            mask_mm(s0, p0, 0)
                            mask_mm(s1, p1, 1)
                            mask_mm(s1, p1, 0)
                            mask_mm(s0, p0, 1)
                        else:
                            msk = m0_sb if s0 == 0 else m1_sb
                            nc.tensor.matmul(sg[:, p0:p0 + 128], msk[:],
                                             ident_sb[:], start=False,
                                             stop=True)
                    elif pair:
                        for (s, kb, qs, w, pos) in pair:
                            msk = m0_sb if s == 0 else m1_sb
                            nc.tensor.matmul(sg[:, pos:pos + 128], msk[:],
                                             ident_sb[:], start=False,
                                             stop=True)
                p_sb = pbp.tile([128, width], F16, tag="pb",
                                name=f"p{qt}_{ci}", padded_shape=[128, 1024])
                nc.scalar.activation(p_sb[:], sg[:], EXP, scale=float(SCALE))
                return p_sb

            def emit_av(qt, ci, p_sb):
                if qt not in o_state:
                    o_state[qt] = psO.tile([H + 1, 512], F32, tag="o",
                                           name=f"o{qt}")
                o_ps = o_state[qt]
                ch, width = qchunks[qt][ci]
                nch = len(qchunks[qt])
                for j, (s, kb, qs, w, pos) in enumerate(ch):
                    nc.tensor.matmul(
                        o_ps[:, qs:512], vcomb[:, kb, s, 0:H + 1],
                        p_sb[:, pos:pos + w],
                        start=(ci == 0 and j == 0),
                        stop=(ci == nch - 1 and j == len(ch) - 1))
                if ci == nch - 1:
                    o_sb = obp.tile([H + 1, 512], F32, name=f"osb{qt}")
                    nc.vector.tensor_copy(o_sb[:], o_ps[:])
                    nc.sync.dma_start(
                        out=outT_ap[:, qt * 512:(qt + 1) * 512], in_=o_sb[:])

            for g_ in proj_groups(0):
                g_()
            from collections import deque
            pend = deque()
            for gi, (qt, ci) in enumerate(glob):
                p = emit_scores(qt, ci)
                pend.append((qt, ci, p))
                for g in slot_groups.get(gi, []):
                    g()
                if len(pend) > LAG:
                    emit_av(*pend.popleft())
            while pend:
                emit_av(*pend.popleft())

    nc.compile()
    return nc


def _get_program():
    if "nc" not in _CACHE:
        _CACHE["nc"] = _build_program()
    return _CACHE["nc"]


def _swizzle_x(xp):
    """[2048, 1024] fp16 parity slice -> [128, NQT, NE, 512] (p, t, e, m)."""
    return np.ascontiguousarray(
        xp.reshape(NQT, 512, NE, 128).transpose(3, 0, 2, 1))


def _swizzle_w(w):
    """[C, n] -> [128, NE*n] (p, (e j))."""
    n = w.shape[1]
    return np.ascontiguousarray(
        w.reshape(NE, 128, n).transpose(1, 0, 2)).reshape(128, NE * n)


def kernel(x, Wk, Wq, Wv, i, embed_dim, head_size_sel, **_unused):
    from concourse import bass_utils

    x = np.asarray(x, dtype=np.float32)
    Wk = np.asarray(Wk, dtype=np.float32)
    Wq = np.asarray(Wq, dtype=np.float32)
    Wv = np.asarray(Wv, dtype=np.float32)

    nc = _get_program()

    idx = np.arange(128)
    m_incl = np.where(idx[None, :] > idx[:, None], NEG, 0.0).astype(np.float16)
    m_strict = np.where(idx[None, :] >= idx[:, None], NEG, 0.0).astype(np.float16)
    ident = np.eye(128, dtype=np.float16)

    wkq = _swizzle_w(np.concatenate([Wk.T, Wq.T], axis=1).astype(np.float16))
    wv_t = _swizzle_w(np.ascontiguousarray(Wv.T).astype(np.float16))
    xh = x.astype(np.float16)

    def cmask(h):
        # s-slots are key-parity slots: m0 masks parity-h==key-parity-0
        # keys, m1 parity-1 keys. For queries of parity h, key parity p is
        # strict iff p > h (key index after query at equal block offset).
        cm = np.zeros((128, CM_COLS), dtype=np.float16)
        cm[:, OFF_M0:OFF_M0 + 128] = m_incl
        cm[:, OFF_M1:OFF_M1 + 128] = m_strict if h == 0 else m_incl
        cm[:, OFF_ID:OFF_ID + 128] = ident
        return cm

    cm0, cm1 = cmask(0), cmask(1)
    in_maps = []
    for c in range(NCORES):
        b, h = c // 2, c % 2
        in_maps.append({
            "xt0": _swizzle_x(xh[b, h::2, :]),
            "wkq": wkq,
            "cmask": cm0 if h == 0 else cm1,
            "wv": wv_t,
        })

    res = bass_utils.run_bass_kernel_spmd(nc, in_maps,
                                          core_ids=list(range(NCORES)))
    _CACHE["last_result"] = res

    out = np.empty((B, T, H), dtype=np.float32)
    for c in range(NCORES):
        b, h = c // 2, c % 2
        outT = res.results[c]["outT"]
        num = outT[:H, :]
        den = outT[H, :]
        out[b, h::2, :] = (num / den[None, :]).T
    return out



# revision 32
# speedup vs baseline: 1.7176x; 1.7176x over previous
"""Trainium2 Bass kernel for single-head causal attention.

Problem: x [4, 4096, 1024], Wk/Wq/Wv [64, 1024] -> out [4, 4096, 64]
  k/q/v = x @ W^T;  out = softmax(causal(q k^T / 8)) @ v

Sharding (8 cores): 2 cores per batch, striped (parity) sequence-parallel
over T. Core c handles batch c//2, query rows of parity c%2. Each core
computes K/V for both parities and full attention for its 2048 query rows.

Structure (v3):
- Row-tiled scores: K for parity0 lives in SBUF partitions 0:64, parity1
  in 64:128; Q is duplicated into both partition halves. The two scores
  matmuls of a (kb, parity-pair) then occupy disjoint PE row-groups
  (tile_position (0,0)/(64,0)) and stream concurrently -- one column
  read of qTd feeds both -- halving scores PE time vs zero-padding.
- exp chunks are [128, 1024] PSUM tiles (2 banks): per-parity blocks are
  first-fit packed into 512-col banks (a matmul output may not cross a
  bank) and mirrored across parities, so each chunk holds one bank-pair;
  chunks alternate two pool tags for double buffering.
- Causal masking of diagonal blocks is fused into scores as an additive
  (-30000) matrix applied via a PE matmul with identity moving operand.
- Startup: the K|Q weights ship first (tiny DMA), the first x tiles are
  split into half-tile DMAs so the first projection starts ~4us earlier;
  remaining consts ship as one contiguous blob; x is pre-swizzled on
  host so every x DMA is a contiguous 8KB-per-partition transfer.
- V gets a ones column so the softmax denominator accumulates in the
  same PSUM tile as the output; host does the final divide.
- Projection work for tile t+1 is interleaved between attention chunks
  of q-tile t so the PE has work while ACT runs exp.
- PSUM: psP 2x1 bank (proj accum + V transposes), psS 2+2 banks
  (score chunks), psO 2 banks (split output accumulators) = 8 banks.

v4 changes:
- AV matmuls are row-tiled: keys 0:64 accumulate into o_ps[:, 0:512]
  (PE rows 0:64), keys 64:128 into o_ps[:, 512:1024] (rows 64:128).
  The two streams share each p_sb column read and run concurrently,
  halving AV PE time; a single DVE add fuses the two accumulators into
  o_sb (replacing the old copy).
- Mask matmuls are row-tiled via identity blocks: mask cols 0:64 from
  contraction rows 0:64 (lhsT=msk[0:64,:], rhs=I64 at parts 0:64) and
  cols 64:128 from rows 64:128, concurrent, halving mask PE time.
- ~8 warm-up matmuls on a memset SBUF tile run during the initial DMA
  latency window so the PE HAM clock gate reaches 2.4 GHz before the
  first projection (was cold 1.2 GHz until ~20us).
"""

import numpy as np

B, T, C, H = 4, 4096, 1024, 64
NCORES = 8
TL = T // 2          # local query rows per core
NB = TL // 128       # 16 local 128-row blocks
NQT = TL // 512      # 4 q-tiles of 512
NE = C // 128        # 8 contraction chunks
SCALE = 1.0 / np.sqrt(H)
NEG = -30000.0

# consts: cmask carries masks+identity (tiny, shipped first on the scalar
# HWDGE queue); cw carries the KV/V projection weights.
OFF_M0, OFF_M1, OFF_ID = 0, 128, 256
CM_COLS = 384
OFF_WKV, OFF_WV = 0, 1024
CW_COLS = 1536

_CACHE = {}


def _unit_plan(qt):
    """FFD-pack one parity's (kb) blocks into 512-col banks; mirror across
    parities. Returns a list of units; each unit is a list of
    (s, kb, qs, w, off) with s=0 blocks at off and s=1 mirrored at 512+off,
    pairs (s0, s1) of the same kb adjacent.
    """
    blocks = []
    nkb = 4 * qt + 4
    for kb in range(nkb):
        qs = max(0, (kb - 4 * qt) * 128)
        blocks.append((kb, qs, 512 - qs))
    blocks.sort(key=lambda b: -b[2])
    banks = []
    for (kb, qs, w) in blocks:
        for bnk in banks:
            if bnk[0] + w <= 512:
                bnk[1].append((kb, qs, w, bnk[0]))
                bnk[0] += w
                break
        else:
            banks.append([w, [(kb, qs, w, 0)]])
    units = []
    for fill, entries in banks:
        unit = []
        for (kb, qs, w, off) in entries:
            unit.append((0, kb, qs, w, off))
            unit.append((1, kb, qs, w, 512 + off))
        units.append((unit, 512 + fill))
    return units


def _build_program():
    import concourse.bacc as bacc
    import concourse.tile as tile
    import concourse.mybir as mybir

    F32 = mybir.dt.float32
    F16 = mybir.dt.float16
    EXP = mybir.ActivationFunctionType.Exp

    nc = bacc.Bacc("TRN2", target_bir_lowering=False, debug=False,
                   num_devices=NCORES)

    xt_ap = [nc.dram_tensor(f"xt{p}", [128, NQT, NE, 512], F16,
                            kind="ExternalInput").ap() for p in (0, 1)]
    wkq_ap = nc.dram_tensor("wkq", [128, NE * 128], F16,
                            kind="ExternalInput").ap()
    cm_ap = nc.dram_tensor("cmask", [128, CM_COLS], F16,
                           kind="ExternalInput").ap()
    cw_ap = nc.dram_tensor("cw", [128, CW_COLS], F16,
                           kind="ExternalInput").ap()
    outT_ap = nc.dram_tensor("outT", [H + 1, TL], F32, kind="ExternalOutput").ap()

    with tile.TileContext(nc) as tc:
        with (
            tc.tile_pool(name="consts", bufs=1) as consts,
            tc.tile_pool(name="persist", bufs=1) as persist,
            tc.tile_pool(name="xin", bufs=8) as xin,
            tc.tile_pool(name="vt", bufs=4) as vtp,
            tc.tile_pool(name="pb", bufs=4) as pbp,
            tc.tile_pool(name="ob", bufs=2) as obp,
            tc.tile_pool(name="psP", bufs=3, space="PSUM") as psP,
            tc.tile_pool(name="psS", bufs=1, space="PSUM") as psS,
            tc.tile_pool(name="psO", bufs=1, space="PSUM") as psO,
        ):
            # ---- weight DMAs on the scalar HWDGE queue (runs concurrently
            # with the x queue on sync): masks first (tiny, unblocks the
            # qt0 diag chain), then K|Q weights, then K|V / V weights.
            cmsk = consts.tile([128, CM_COLS], F16)
            nc.scalar.dma_start(cmsk[:], cm_ap[:])
            wkqf = consts.tile([128, NE * 128], F16)
            nc.scalar.dma_start(wkqf[:], wkq_ap[:])
            cwt = consts.tile([128, CW_COLS], F16)
            nc.scalar.dma_start(cwt[:], cw_ap[:])

            # warm tile for ACT table preload (no DMA dependency)
            warm_sb = consts.tile([128, 8], F16)
            escr = consts.tile([128, 1], F16)
            nc.vector.memset(warm_sb[:], 1.0)
            nc.scalar.activation(escr[:], warm_sb[:, 0:1], EXP, scale=1.0)

            # PE HAM warm-up: dummy matmuls on a memset tile fill the
            # initial DMA-latency window so the clock gate opens (2.4GHz)
            # before the first real projection.
            NDUM = 10
            warm_mm = consts.tile([128, 512], F16)
            nc.vector.memset(warm_mm[:], 0.001)
            dum_ps = psP.tile([128, 512], F32, tag="p", name="dummy")
            for _ in range(NDUM):
                nc.tensor.matmul(dum_ps[:], warm_mm[:, 0:128], warm_mm[:],
                                 start=True, stop=True)

            # ---- x DMAs on the sync HWDGE queue ----
            x_sb = {}

            def emit_xdma(p, t, split):
                xs = xin.tile([128, NE, 512], F16, tag="x", name=f"x{p}_{t}")
                if split:
                    nc.sync.dma_start(out=xs[:, 0:2, :],
                                      in_=xt_ap[p][:, t, 0:2])
                    nc.sync.dma_start(out=xs[:, 2:4, :],
                                      in_=xt_ap[p][:, t, 2:4])
                    nc.sync.dma_start(out=xs[:, 4:8, :],
                                      in_=xt_ap[p][:, t, 4:8])
                else:
                    nc.sync.dma_start(out=xs[:], in_=xt_ap[p][:, t])
                x_sb[(p, t)] = xs

            emit_xdma(0, 0, True)
            emit_xdma(1, 0, True)
            for t in range(1, NQT):
                for p in (0, 1):
                    emit_xdma(p, t, False)

            wkq_sb = wkqf.rearrange("p (e j) -> p e j", e=NE)
            wkv_sb = cwt[:, OFF_WKV:OFF_WKV + 1024].rearrange(
                "p (e j) -> p e j", e=NE)
            wv_sb = cwt[:, OFF_WV:OFF_WV + 512].rearrange(
                "p (e j) -> p e j", e=NE)
            m0_sb = cmsk[:, OFF_M0:OFF_M0 + 128]
            m1_sb = cmsk[:, OFF_M1:OFF_M1 + 128]
            ident_sb = cmsk[:, OFF_ID:OFF_ID + 128]

            # ---- persistent K/Q/V ----
            # kTrt rows 0:64 = K parity0, rows 64:128 = K parity1
            # qTd: Q duplicated in both partition halves
            kTrt = persist.tile([128, TL], F16)
            qTd = persist.tile([128, TL], F16)
            vcomb = persist.tile([128, NB, 2, H + 1], F16)
            nc.vector.memset(vcomb[:, :, :, H:H + 1], 1.0)

            # ---- projection groups ----
            def g_kq(t):
                cols = slice(t * 512, (t + 1) * 512)
                x0 = x_sb[(0, t)]
                kq_ps = psP.tile([128, 512], F32, tag="p", name=f"kq{t}")
                for e in range(NE):
                    nc.tensor.matmul(kq_ps[:], wkq_sb[:, e, :], x0[:, e, :],
                                     start=(e == 0), stop=(e == NE - 1))
                nc.vector.tensor_copy(kTrt[0:64, cols], kq_ps[0:64, :])
                nc.vector.tensor_copy(qTd[0:64, cols], kq_ps[64:128, :])
                nc.vector.tensor_copy(qTd[64:128, cols], kq_ps[64:128, :])

            def g_kv(t):
                cols = slice(t * 512, (t + 1) * 512)
                x1 = x_sb[(1, t)]
                kv_ps = psP.tile([128, 512], F32, tag="p", name=f"kv{t}")
                for e in range(NE):
                    nc.tensor.matmul(kv_ps[:], wkv_sb[:, e, :], x1[:, e, :],
                                     start=(e == 0), stop=(e == NE - 1))
                nc.vector.tensor_copy(kTrt[64:128, cols], kv_ps[0:64, :])
                vst1 = vtp.tile([128, 256], F16, tag="v", name=f"vs1_{t}")
                nc.vector.tensor_copy(vst1[0:64, :], kv_ps[64:128, 0:256])
                nc.vector.tensor_copy(vst1[64:128, :], kv_ps[64:128, 256:512])
                return vst1

            def g_v0(t):
                # The two 64-channel halves go to disjoint PSUM partition
                # ranges (col groups h0/h64); interleaving their emission
                # lets each e-pair run concurrently on the PE (full
                # contraction, disjoint col groups) -- 2048 instead of
                # 4096 effective columns per tile.
                x0 = x_sb[(0, t)]
                v0_ps = psP.tile([128, 256], F32, tag="p", name=f"v0{t}")
                for e in range(NE):
                    nc.tensor.matmul(v0_ps[0:64, :], wv_sb[:, e, :],
                                     x0[:, e, 0:256],
                                     start=(e == 0), stop=(e == NE - 1))
                    nc.tensor.matmul(v0_ps[64:128, :], wv_sb[:, e, :],
                                     x0[:, e, 256:512],
                                     start=(e == 0), stop=(e == NE - 1))
                vst0 = vtp.tile([128, 256], F16, tag="v", name=f"vs0_{t}")
                nc.vector.tensor_copy(vst0[:], v0_ps[:])
                return vst0

            def g_tp(t, vst0, vst1):
                # chunk u covers blocks 4t+u (cols 0:64), 4t+2+u (cols 64:128)
                for s, vst in ((0, vst0), (1, vst1)):
                    for u in range(2):
                        tp = psP.tile([128, 128], F16, tag="p",
                                      name=f"tp{t}_{s}_{u}")
                        nc.tensor.transpose(tp[:], vst[:, u * 128:(u + 1) * 128],
                                            ident_sb[:])
                        nc.vector.tensor_copy(vcomb[:, 4 * t + u, s, 0:H],
                                              tp[:, 0:64])
                        nc.vector.tensor_copy(vcomb[:, 4 * t + 2 + u, s, 0:H],
                                              tp[:, 64:128])

            def proj_groups(t):
                holder = {}

                def a():
                    g_kq(t)

                def b():
                    holder["v1"] = g_kv(t)

                def c():
                    holder["v0"] = g_v0(t)

                def d():
                    g_tp(t, holder["v0"], holder["v1"])

                return [a, b, c, d]

            # ---- attention: global exp-ahead pipeline ----
            # scores+exp stream runs ahead; AV matmuls trail by LAG chunks
            # so the ACT engine is never starved waiting for a q-tile's AVs.
            LAG = 2
            tag_state = {"flip": False}
            qchunks = {qt: _unit_plan(qt) for qt in range(NQT)}
            glob = [(qt, ci) for qt in range(NQT)
                    for ci in range(len(qchunks[qt]))]
            # slot proj groups so only kq/kv of tile t precede q-tile t's
            # scores, and tile0's V work comes after the first score chunks:
            # kq(t) @ (t-1, nch-2), kv(t) @ (t-1, nch-1), v0/tp(t) @ (t, 0/1)
            slot_groups = {}
            for t in range(1, NQT):
                a, b_, c, d_ = proj_groups(t)
                prev_n = len(qchunks[t - 1])
                slot_groups.setdefault(
                    glob.index((t - 1, prev_n - 2)), []).append(a)
                slot_groups.setdefault(
                    glob.index((t - 1, prev_n - 1)), []).append(b_)
                slot_groups.setdefault(glob.index((t, 0)), []).append(c)
                slot_groups.setdefault(glob.index((t, 1)), []).append(d_)

            o_state = {}

            def emit_scores(qt, ci):
                ch, width = qchunks[qt][ci]
                tag = "sA" if tag_state["flip"] else "sB"
                tag_state["flip"] = not tag_state["flip"]
                sg = psS.tile([128, width], F32, tag=tag,
                              name=f"sg{qt}_{ci}", padded_shape=[128, 1024])
                for j in range(0, len(ch), 2):
                    for (s, kb, qs, w, pos) in ch[j:j + 2]:
                        rows = slice(64 * s, 64 * s + 64)
                        diag = kb >= 4 * qt
                        nc.tensor.matmul(
                            sg[:, pos:pos + w],
                            kTrt[rows, kb * 128:(kb + 1) * 128],
                            qTd[rows, qt * 512 + qs:(qt + 1) * 512],
                            start=True, stop=not diag)
                    pair = [b for b in ch[j:j + 2] if b[1] >= 4 * qt]
                    if pair and False:  # bisect: mask pairing disabled
                        # Row-tiled mask adds. Each mask splits into a
                        # rows-0:64 half (cols 0:64 via I64 at parts 0:64)
                        # and a rows-64:128 half (cols 64:128). Pair the
                        # s0 A-half with the s1 B-half (and vice versa) so
                        # each concurrent pair hits disjoint PE row groups
                        # AND disjoint PSUM banks (s0 is in bank 0, s1 in
                        # bank 1 of the chunk).
                        def mask_mm(s, pos, half):
                            msk = m0_sb if s == 0 else m1_sb
                            if half == 0:
                                nc.tensor.matmul(sg[:, pos:pos + 64],
                                                 msk[0:64, :],
                                                 ident_sb[0:64, 0:64],
                                                 start=False, stop=True)
                            else:
                                nc.tensor.matmul(sg[:, pos + 64:pos + 128],
                                                 msk[64:128, :],
                                                 ident_sb[64:128, 64:128],
                                                 start=False, stop=True)

                        (s0, _, _, _, p0) = pair[0]
                        if len(pair) == 2:
                            (s1, _, _, _, p1) = pair[1]
                            mask_mm(s0, p0, 0)
                            mask_mm(s1, p1, 1)
                            mask_mm(s1, p1, 0)
                            mask_mm(s0, p0, 1)
                        else:
                            msk = m0_sb if s0 == 0 else m1_sb
                            nc.tensor.matmul(sg[:, p0:p0 + 128], msk[:],
                                             ident_sb[:], start=False,
                                             stop=True)
                    elif pair:
                        for (s, kb, qs, w, pos) in pair:
                            msk = m0_sb if s == 0 else m1_sb
                            nc.tensor.matmul(sg[:, pos:pos + 128], msk[:],
                                             ident_sb[:], start=False,
                                             stop=True)
                p_sb = pbp.tile([128, width], F16, tag="pb",
                                name=f"p{qt}_{ci}", padded_shape=[128, 1024])
                nc.scalar.activation(p_sb[:], sg[:], EXP, scale=float(SCALE))
                return p_sb

            def emit_av(qt, ci, p_sb):
                if qt not in o_state:
                    o_state[qt] = psO.tile([H + 1, 512], F32, tag="o",
                                           name=f"o{qt}")
                o_ps = o_state[qt]
                ch, width = qchunks[qt][ci]
                nch = len(qchunks[qt])
                for j, (s, kb, qs, w, pos) in enumerate(ch):
                    nc.tensor.matmul(
                        o_ps[:, qs:512], vcomb[:, kb, s, 0:H + 1],
                        p_sb[:, pos:pos + w],
                        start=(ci == 0 and j == 0),
                        stop=(ci == nch - 1 and j == len(ch) - 1))
                if ci == nch - 1:
                    o_sb = obp.tile([H + 1, 512], F32, name=f"osb{qt}")
                    nc.vector.tensor_copy(o_sb[:], o_ps[:])
                    nc.sync.dma_start(
                        out=outT_ap[:, qt * 512:(qt + 1) * 512], in_=o_sb[:])

            g0 = proj_groups(0)
            g0[0]()
            g0[1]()
            for gi_, g_ in ((0, g0[2]), (1, g0[3])):
                slot_groups.setdefault(gi_, []).append(g_)
            from collections import deque
            pend = deque()
            for gi, (qt, ci) in enumerate(glob):
                p = emit_scores(qt, ci)
                pend.append((qt, ci, p))
                for g in slot_groups.get(gi, []):
                    g()
                if len(pend) > LAG:
                    emit_av(*pend.popleft())
            while pend:
                emit_av(*pend.popleft())

    nc.compile()
    return nc


def _get_program():
    if "nc" not in _CACHE:
        _CACHE["nc"] = _build_program()
    return _CACHE["nc"]


def _swizzle_x(xp):
    """[2048, 1024] fp16 parity slice -> [128, NQT, NE, 512] (p, t, e, m)."""
    return np.ascontiguousarray(
        xp.reshape(NQT, 512, NE, 128).transpose(3, 0, 2, 1))


def _swizzle_w(w):
    """[C, n] -> [128, NE*n] (p, (e j))."""
    n = w.shape[1]
    return np.ascontiguousarray(
        w.reshape(NE, 128, n).transpose(1, 0, 2)).reshape(128, NE * n)


def kernel(x, Wk, Wq, Wv, i, embed_dim, head_size_sel, **_unused):
    from concourse import bass_utils

    x = np.asarray(x, dtype=np.float32)
    Wk = np.asarray(Wk, dtype=np.float32)
    Wq = np.asarray(Wq, dtype=np.float32)
    Wv = np.asarray(Wv, dtype=np.float32)

    nc = _get_program()

    idx = np.arange(128)
    m_incl = np.where(idx[None, :] > idx[:, None], NEG, 0.0).astype(np.float16)
    m_strict = np.where(idx[None, :] >= idx[:, None], NEG, 0.0).astype(np.float16)
    ident = np.eye(128, dtype=np.float16)

    wkq = _swizzle_w(np.concatenate([Wk.T, Wq.T], axis=1).astype(np.float16))
    wkv = _swizzle_w(np.concatenate([Wk.T, Wv.T], axis=1).astype(np.float16))
    wv_t = _swizzle_w(np.ascontiguousarray(Wv.T).astype(np.float16))
    xh = x.astype(np.float16)

    def cmask(h):
        cm = np.zeros((128, CM_COLS), dtype=np.float16)
        cm[:, OFF_M0:OFF_M0 + 128] = m_incl
        cm[:, OFF_M1:OFF_M1 + 128] = m_strict if h == 0 else m_incl
        cm[:, OFF_ID:OFF_ID + 128] = ident
        return cm

    cw = np.zeros((128, CW_COLS), dtype=np.float16)
    cw[:, OFF_WKV:OFF_WKV + 1024] = wkv
    cw[:, OFF_WV:OFF_WV + 512] = wv_t

    cm0, cm1 = cmask(0), cmask(1)
    in_maps = []
    for c in range(NCORES):
        b, h = c // 2, c % 2
        in_maps.append({
            "xt0": _swizzle_x(xh[b, h::2, :]),
            "xt1": _swizzle_x(xh[b, 1 - h::2, :]),
            "wkq": wkq,
            "cmask": cm0 if h == 0 else cm1,
            "cw": cw,
        })

    res = bass_utils.run_bass_kernel_spmd(nc, in_maps,
                                          core_ids=list(range(NCORES)))
    _CACHE["last_result"] = res

    out = np.empty((B, T, H), dtype=np.float32)
    for c in range(NCORES):
        b, h = c // 2, c % 2
        outT = res.results[c]["outT"]
        num = outT[:H, :]
        den = outT[H, :]
        out[b, h::2, :] = (num / den[None, :]).T
    return out

